# revision 14
# baseline (speedup 1.0000x reference)
"""RBF kernel layer (retrieval_knn): out = exp(-||x - p||^2) for x [131072, 64]
against 512 prototypes, distributed data-parallel over 8 NeuronCores.

Math: exp(-dist2) = exp(2*S) where S[n,m] = cross[n,m] - p_sq[m]/2 - x_sq[n]/2,
computed entirely in two bf16 hi/lo-split GEMMs accumulating in fp32 PSUM:
  mm1: [xh_t; nxsq_h; nxsq_l; 1; 1].T @ [ph; 1; 1; npsq_h; npsq_l]  (K=68)
  mm2: [xh_t; xl_t].T @ [pl; ph]                                    (K=128)
where x = xh + xl, p = ph + pl (bf16 splits; the dropped xl@pl term is
~2^-18), npsq* = bf16 split of -p_sq/2, nxsq* = bf16 split of -x_sq/2.

Perf structure (v8):
- x loaded as 16 independent 256 KB chunk tiles, X0 issued before the
  small tables, so matmuls start right after the first chunk lands
  instead of after the full 4 MB load.
- nxsq ([4, nshard] on partitions 0-3, all served by one DMA engine) is
  loaded in 4 column chunks so tile 0's A-copy waits only for the first
  32 KB of it, not all 128 KB.
- Output DRAM layout is partition-major [128, nt, M] (host transposes
  back): per-partition store data is contiguous -> 16 KB descriptors.
  SCHUNK tiles per store DMA (2 MB), staged in SBUF by the EXP
  activation itself. All store/compute APs keep the full 128 partitions:
  the SDMA engine dealing is only balanced for 128-partition APs.
- Stores issue from nc.scalar (ACT's HWDGE ring): EXP -> store is
  same-engine program order; loads on nc.sync never queue behind them.
- Deep staging (STG_BUFS) decouples the producer from the slowest DMA
  engine so no SDMA engine idles waiting for queued work.
"""

import numpy as np

# Problem constants (hardcoded per harness contract; kernel.py is self-contained)
N = 131072
D = 64
M = 512
GAMMA = 1.0
NCORES = 8
NSHARD = N // NCORES  # 16384
P = 128
K1 = D + 4  # mm1 contraction: 64 xh rows + 2 xsq rows + 2 ones rows
LHS_SLOTS = 4  # manual rotation slots for A
XCHUNK = 8  # x tiles per input chunk DMA (256 KB)
XBUFS = 5  # x chunk buffers: chunk c's load waits for chunk c-5's reads,
#            spreading the 4 MB load across the run as DMA gap-filler
OCHUNK = 4  # output tiles per ACTIVATE (PSUM 4-bank group)
SCHUNK = 8  # output tiles per store DMA (2 MB)
STG_BUFS = 6  # staging buffers in flight

_cache = {}


def _build_bass(nshard=NSHARD):
    import concourse.mybir as mybir
    import concourse.tile as tile
    from concourse import bacc

    f32 = mybir.dt.float32
    bf16 = mybir.dt.bfloat16
    nt = nshard // P
    assert nt % XCHUNK == 0 and nt % SCHUNK == 0 and SCHUNK % OCHUNK == 0

    nc = bacc.Bacc(None, target_bir_lowering=False)
    # pre-transposed on host: [p, i*P + j] = [xh|xl] feature p of point i*P+j
    xhl_d = nc.dram_tensor("xhl", [P, nshard], bf16, kind="ExternalInput")
    # nxsq rows (-x_sq/2 hi, lo) for tile i: DRAM rows 2*(i%4), +1,
    # cols (i//4)*P (loaded to SBUF quadrant starts {0,32,64,96}: DVE copy
    # sources must be 32-aligned; spreads the load over two DMA engines)
    nxsq_d = nc.dram_tensor("nxsq", [8, (nt // 4) * P], bf16, kind="ExternalInput")
    ones_d = nc.dram_tensor("ones2", [2, P], bf16, kind="ExternalInput")
    # rhs2 in cols 0..511 (128 rows), rhs1 in cols 512..1023 (rows 0..67):
    # one full-128-partition DMA keeps its descriptors balanced over all
    # 16 SDMA engines instead of piling 1 KB descriptors on engines 0-3
    rhs_d = nc.dram_tensor("rhs", [P, 2 * M], bf16, kind="ExternalInput")
    # partition-major: out_d[p, t, m] = out row t*P+p, col m (host transposes)
    out_d = nc.dram_tensor("out", [P, nt, M], f32, kind="ExternalOutput")

    with tile.TileContext(nc) as tc:
        with (
            tc.tile_pool(name="singles", bufs=1) as singles,
            tc.tile_pool(name="xp", bufs=XBUFS) as xpool,
            tc.tile_pool(name="stg", bufs=STG_BUFS) as stgp,
            tc.tile_pool(name="ps_o", bufs=2, space="PSUM") as ps_o,
        ):
            # x chunks from a rotating pool: chunk c reuses chunk c-XBUFS's
            # buffer, so its DMA is held (WAR) until that chunk is consumed.
            # This paces the 4 MB x load across the whole run instead of
            # draining it in the first ~15 us, keeping the SDMA engines fed
            # through the window before the store backlog is deep enough.
            x_tiles = []

            def load_x_chunk(c):
                cs = slice(c * XCHUNK * P, (c + 1) * XCHUNK * P)
                xt = xpool.tile([P, XCHUNK * P], bf16, tag="x")
                nc.sync.dma_start(xt[:], xhl_d[:, cs])
                x_tiles.append(xt)

            for c in range(min(XBUFS, nt // XCHUNK)):
                load_x_chunk(c)

            rhs_sb = singles.tile([P, 2 * M], bf16)
            nc.scalar.dma_start(rhs_sb[:], rhs_d[:])
            rhs1_sb = rhs_sb[0:K1, M : 2 * M]
            rhs2_sb = rhs_sb[:, 0:M]

            # A slots [68, 128]: rows 0..63 = xh_t, 64..65 = per-tile
            # [-x_sq/2 hi; lo], 66..67 = ones (constant, DMA'd once: DVE
            # cannot write at partition 66, DMA can).
            a_slots = []
            for j in range(LHS_SLOTS):
                A_sb = singles.tile([K1, P], bf16, name=f"A{j}")
                nc.scalar.dma_start(A_sb[D + 2 : K1, :], ones_d[:])
                a_slots.append(A_sb)

            # nxsq at SBUF quadrant starts {0,32,64,96}, 2 rows each
            nxsq_sb = singles.tile([P, (nt // 4) * P], bf16)
            for q in range(4):
                nc.scalar.dma_start(
                    nxsq_sb[32 * q : 32 * q + 2, :],
                    nxsq_d[2 * q : 2 * q + 2, :],
                )

            nxc = nt // XCHUNK
            for i in range(nt):
                c, col = divmod(i, XCHUNK)
                if col == 0 and c + XBUFS < nxc:
                    load_x_chunk(c + XBUFS)
                k = i % OCHUNK
                j = i % SCHUNK
                if k == 0:
                    psum = ps_o.tile([P, OCHUNK, M], f32, tag="psum")
                if j == 0:
                    stg = stgp.tile([P, SCHUNK, M], f32, tag="stg")

                Xc = x_tiles[c]
                T = Xc[:, col * P : (col + 1) * P]
                A = a_slots[i % LHS_SLOTS]
                nc.vector.tensor_copy(A[0:D, :], T[0:D, :])
                a0 = 32 * (i % 4)
                c0 = (i // 4) * P
                nc.vector.tensor_copy(
                    A[D : D + 2, :], nxsq_sb[a0 : a0 + 2, c0 : c0 + P]
                )
                nc.tensor.matmul(
                    psum[:, k, :], A[:], rhs1_sb, start=True, stop=False
                )
                nc.tensor.matmul(
                    psum[:, k, :], T, rhs2_sb, start=False, stop=True
                )

                if k == OCHUNK - 1:
                    # out = exp(2*S) over all OCHUNK PSUM banks at once,
                    # written straight into the staging slot for the store
                    g = j // OCHUNK
                    nc.scalar.activation(
                        stg[:, g * OCHUNK : (g + 1) * OCHUNK, :],
                        psum[:],
                        mybir.ActivationFunctionType.Exp,
                        bias=0.0,
                        scale=2.0,
                    )
                    if j == SCHUNK - 1:
                        i0 = i - (SCHUNK - 1)
                        nc.scalar.dma_start(
                            out_d[:, i0 : i0 + SCHUNK, :], stg[:]
                        )

    nc.finalize()
    return nc


def _get_nc():
    if "nc" not in _cache:
        _cache["nc"] = _build_bass()
    return _cache["nc"]


def _prep_core_arrays(x, prototypes, nshard):
    """Build per-core host arrays (xhl row-major, nxsq, rhs1/rhs2)."""
    import ml_dtypes

    bf = ml_dtypes.bfloat16
    x = np.ascontiguousarray(np.asarray(x, dtype=np.float32))
    prototypes = np.ascontiguousarray(np.asarray(prototypes, dtype=np.float32))

    xh = x.astype(bf)
    xl = (x - xh.astype(np.float32)).astype(bf)
    # [128, N]: rows 0..63 = xh features, 64..127 = xl features
    xhl_t = np.ascontiguousarray(
        np.concatenate([xh, xl], axis=1).T
    )

    nxsq = (-0.5 * (x.astype(np.float64) ** 2).sum(axis=1)).astype(np.float32)
    nxh = nxsq.astype(bf)
    nxl = (nxsq - nxh.astype(np.float32)).astype(bf)

    pt = prototypes.T.astype(np.float32)  # [64, 512]
    ph = pt.astype(bf)
    pl = (pt - ph.astype(np.float32)).astype(bf)

    p_sq = (prototypes.astype(np.float64) ** 2).sum(axis=1)  # [512]
    t = (-0.5 * p_sq).astype(np.float32)
    th = t.astype(bf)
    tl = (t - th.astype(np.float32)).astype(bf)

    ones = np.ones((1, M), dtype=bf)
    # row order matches A: [xh_t rows; nxsq h/l rows; ones rows]
    rhs1 = np.ascontiguousarray(
        np.concatenate([ph, ones, ones, th[None, :], tl[None, :]], axis=0)
    )  # [68, 512] bf16
    rhs2 = np.ascontiguousarray(np.concatenate([pl, ph], axis=0))  # [128, 512]

    # merged rhs [128, 1024]: cols 0..511 = rhs2, cols 512..1023 rows
    # 0..67 = rhs1 (rest zero)
    rhs_all = np.zeros((P, 2 * M), dtype=bf)
    rhs_all[:, 0:M] = rhs2
    rhs_all[0:K1, M : 2 * M] = rhs1
    rhs_all = np.ascontiguousarray(rhs_all)
    ones2 = np.ones((2, P), dtype=bf)

    ncores = x.shape[0] // nshard
    nt = nshard // P
    in_maps = []
    for s in range(ncores):
        sl = slice(s * nshard, (s + 1) * nshard)
        nxsq_r = np.stack([nxh[sl], nxl[sl]], axis=0)  # [2, nshard]
        # spread layout [8, (nt//4)*P]: tile i's 2 rows at DRAM rows
        # 2*(i%4)..+1, cols (i//4)*P..
        t2 = nxsq_r.reshape(2, nt, P).transpose(1, 0, 2)  # [nt, 2, P]
        t2 = t2.reshape(nt // 4, 4, 2, P).transpose(1, 2, 0, 3)
        nxsq_spread = np.ascontiguousarray(t2.reshape(8, (nt // 4) * P))
        in_maps.append(
            {
                "xhl": np.ascontiguousarray(xhl_t[:, sl]),
                "nxsq": nxsq_spread,
                "ones2": ones2,
                "rhs": rhs_all,
            }
        )
    return in_maps


def _prep_inputs(x, prototypes):
    return _prep_core_arrays(x, prototypes, NSHARD)


def _run(inputs, trace=False):
    from concourse.bass_utils import run_bass_kernel_spmd

    in_maps = _prep_inputs(inputs["x"], inputs["prototypes"])
    nc = _get_nc()
    res = run_bass_kernel_spmd(
        nc, in_maps, core_ids=list(range(NCORES)), trace=trace
    )
    # out is partition-major [P, nt, M]; row t*P+p of the shard = out[p, t]
    out = np.concatenate(
        [
            r["out"].transpose(1, 0, 2).reshape(NSHARD, M)
            for r in res.results
        ],
        axis=0,
    )
    return np.ascontiguousarray(out), res


def kernel(**inputs) -> np.ndarray:
    out, _ = _run(inputs, trace=False)
    return out
